# revision 10
# baseline (speedup 1.0000x reference)
"""DiT attention (B=2, S=2048, DIM=1024, H=16, D=64) on 8 TRN2 NeuronCores.

Sharding: data-parallel over B (2) x tensor-parallel over head groups (4),
so each core owns (one batch, 4 heads). The out-projection is computed as
per-core partials over the 256 e-channels each core owns; the host sums the
4 partials per batch and adds out_b (cheaper than an on-device all-reduce).

v2 device dataflow (per core, bf16 matmuls, fp32 PSUM):
  - scores use 64-key tiles packed 4-way onto the PE array via tile_position
    (head parity -> row groups, key-tile parity -> col groups): 4 concurrent
    K=64/M=64/N=512 matmuls per 128-key block.
  - AV runs 2-way row-packed (key parity) with the softmax denominator fused
    as weight column 64 (M=65); four PSUM banks accumulate (head x parity),
    merged by one DVE add per head at drain.
  - exp on ScalarE paces the attention phase (~1 elem/cycle is a hard floor);
    everything else (QK proj of the second head pair, drains, normalize) is
    scheduled into the PE/DVE slack under it.
  - normalization deferred: O^T stays unnormalized in SBUF; 1/denom rows are
    broadcast across partitions via a DRAM bounce and applied with one
    in-place DVE multiply per (pair, chunk).
  - out-projection runs as a tail phase reusing the AV PSUM banks, DMAing
    y [s, dim] fp32 straight from PSUM.
"""

import numpy as np
import ml_dtypes

import concourse.bacc as bacc
import concourse.bass as bass
import concourse.mybir as mybir
import concourse.tile as tile
from concourse.bass_utils import run_bass_kernel_spmd

B, S, DIM, H, D = 2, 2048, 1024, 16, 64
NCORES = 8
GROUPS = 4     # head groups (tensor parallel)
HPG = 4        # heads per group
E = HPG * D    # 256 e-channels per core per projection
P = 128        # partitions
SC = 512       # free-dim chunk for matmuls
NKTP = S // P  # 16 key-tile pairs (each = two 64-key halves)
NQC = S // SC  # 4 query chunks
NST = S // P   # 16 s tiles
BF = mybir.dt.bfloat16
F32 = mybir.dt.float32

_BF_NP = ml_dtypes.bfloat16


def _build_nc():
    nc = bacc.Bacc(None, target_bir_lowering=False)

    hT_d = nc.declare_dram_parameter("hT", [DIM, S], BF, isOutput=False)
    wqkvT_d = nc.declare_dram_parameter("wqkvT", [DIM, 3 * E], BF, isOutput=False)
    qkvb_d = nc.declare_dram_parameter("qkvb", [1, 3 * E], BF, isOutput=False)
    qkvbc_d = nc.declare_dram_parameter("qkvb_col", [2 * E, 1], F32, isOutput=False)
    woutT_d = nc.declare_dram_parameter("woutT", [E, DIM], BF, isOutput=False)
    cos_d = nc.declare_dram_parameter("cos_t", [D, S], BF, isOutput=False)
    sin_d = nc.declare_dram_parameter("sin_t", [D, S], BF, isOutput=False)
    perm_d = nc.declare_dram_parameter("perm", [D, D], BF, isOutput=False)
    y_d = nc.declare_dram_parameter("y", [S, DIM], F32, isOutput=True)

    hT_t = hT_d.ap().rearrange("(t p) s -> t p s", p=P)        # [8,128,S]
    wqkvT_t = wqkvT_d.ap().rearrange("(t p) e -> t p e", p=P)  # [8,128,768]
    woutT_t = woutT_d.ap().rearrange("(t p) o -> t p o", p=P)  # [2,128,DIM]

    with tile.TileContext(nc) as tc:
        import contextlib
        with contextlib.ExitStack() as ctx:
            consts = ctx.enter_context(tc.tile_pool(name="consts", bufs=1))
            ptpool = ctx.enter_context(tc.tile_pool(name="ptpool", bufs=2))
            work = ctx.enter_context(tc.tile_pool(name="work", bufs=2))
            dram = ctx.enter_context(tc.tile_pool(name="dram", bufs=2, space="DRAM"))
            psum = ctx.enter_context(
                tc.tile_pool(name="psum", bufs=2, space="PSUM"))

            # ---- persistent SBUF tensors ----
            hT_sb = consts.tile([P, DIM // P, S], BF, name="hT_sb")
            wqkvT_sb = consts.tile([P, DIM // P, 3 * E], BF, name="wqkvT_sb")
            qkvb_sb = consts.tile([1, 3 * E], BF, name="qkvb_sb")
            qkvbc_sb = consts.tile([P, 2 * E // P, 1], F32, name="qkvbc_sb")
            woutT_sb = consts.tile([P, E // P, DIM], BF, name="woutT_sb")
            cos_sb = consts.tile([D, S], BF, name="cos_sb")
            sin_sb = consts.tile([D, S], BF, name="sin_sb")
            perm_sb = consts.tile([D, D], BF, name="perm_sb")
            ones_sb = consts.tile([1, SC], BF, name="ones_sb")
            # V with keys on partitions split by 64-key parity; col 64 = ones
            # (fused softmax-denominator row of the M=65 AV matmuls)
            V_sb = consts.tile([P, NKTP, HPG, D + 1], BF, name="V_sb")
            QT_sb = consts.tile([P, E // P, S], BF, name="QT_sb")
            KT_sb = consts.tile([P, E // P, S], BF, name="KT_sb")
            OT_sb = consts.tile([P, E // P, S], BF, name="OT_sb")
            q0r = consts.tile([D, S], BF, name="q0r")
            k0r = consts.tile([D, S], BF, name="k0r")
            qtmp = consts.tile([D, S], BF, name="qtmp")
            ktmp = consts.tile([D, S], BF, name="ktmp")

            # ---- loads ----
            for t in range(DIM // P):
                nc.sync.dma_start(out=hT_sb[:, t, :], in_=hT_t[t])
                nc.sync.dma_start(out=wqkvT_sb[:, t, :], in_=wqkvT_t[t])
            nc.sync.dma_start(out=qkvb_sb[:, :], in_=qkvb_d.ap())
            for t in range(2 * E // P):
                nc.sync.dma_start(out=qkvbc_sb[:, t, :],
                                  in_=qkvbc_d.ap()[t * P:(t + 1) * P, :])
            for t in range(E // P):
                nc.sync.dma_start(out=woutT_sb[:, t, :], in_=woutT_t[t])
            nc.sync.dma_start(out=cos_sb[:, :], in_=cos_d.ap())
            nc.sync.dma_start(out=sin_sb[:, :], in_=sin_d.ap())
            nc.sync.dma_start(out=perm_sb[:, :], in_=perm_d.ap())
            nc.vector.memset(ones_sb[:, :], 1.0)
            nc.vector.memset(V_sb[:, :, :, D:D + 1], 1.0)

            # ---- V projection: V[s, e] = hT.T @ Wv^T + bias (K=1 matmul) ----
            for st in range(NST):
                v_ps = psum.tile([P, E], F32, name="v_ps", tag="s_ps")
                for kt in range(DIM // P):
                    nc.tensor.matmul(
                        out=v_ps[:, :],
                        lhsT=hT_sb[:, kt, st * P:(st + 1) * P],
                        rhs=wqkvT_sb[:, kt, 2 * E:3 * E],
                        start=(kt == 0), stop=False)
                nc.tensor.matmul(
                    out=v_ps[:, :],
                    lhsT=ones_sb[0:1, 0:P],
                    rhs=qkvb_sb[0:1, 2 * E:3 * E],
                    start=False, stop=True)
                nc.vector.tensor_copy(
                    out=V_sb[:, st, :, 0:D],
                    in_=v_ps[:, :].rearrange("p (h c) -> p h c", h=HPG))

            # ---- Q^T/K^T projection for one head pair (e-slab) ----
            def qk_proj(pair):
                for which, dst in ((0, QT_sb), (1, KT_sb)):
                    ecols = slice(which * E + pair * P, which * E + (pair + 1) * P)
                    for scn in range(NQC):
                        s_sl = slice(scn * SC, (scn + 1) * SC)
                        qk_ps = psum.tile([P, SC], F32, name="qk_ps", tag="s_ps")
                        for kt in range(DIM // P):
                            nc.tensor.matmul(
                                out=qk_ps[:, :],
                                lhsT=wqkvT_sb[:, kt, ecols],
                                rhs=hT_sb[:, kt, s_sl],
                                start=(kt == 0), stop=(kt == DIM // P - 1))
                        nc.vector.tensor_scalar_add(
                            out=dst[:, pair, s_sl], in0=qk_ps[:, :],
                            scalar1=qkvbc_sb[:, which * 2 + pair, :])

            qk_proj(0)

            # ---- RoPE on local head 0 (identity tables on non-rope cores):
            # PSUM-reading ops stay on DVE; SBUF-only ops of the k-path go to
            # the otherwise-idle GpSimd so DVE doesn't serialize phase A ----
            nc.vector.tensor_mul(out=qtmp[:, :], in0=QT_sb[0:D, 0, :], in1=cos_sb[:, :])
            nc.gpsimd.tensor_mul(out=ktmp[:, :], in0=KT_sb[0:D, 0, :], in1=cos_sb[:, :])
            for src, dst, tmp, add_eng in ((QT_sb, q0r, qtmp, nc.vector),
                                           (KT_sb, k0r, ktmp, nc.gpsimd)):
                for scn in range(NQC):
                    s_sl = slice(scn * SC, (scn + 1) * SC)
                    sw_ps = psum.tile([D, SC], F32, name="sw_ps", tag="s_ps")
                    nc.tensor.matmul(
                        out=sw_ps[:, :], lhsT=perm_sb[:, :],
                        rhs=src[0:D, 0, s_sl], start=True, stop=True)
                    nc.vector.tensor_mul(
                        out=dst[:, s_sl], in0=sw_ps[:, :], in1=sin_sb[:, s_sl])
                    add_eng.tensor_add(
                        out=dst[:, s_sl], in0=dst[:, s_sl], in1=tmp[:, s_sl])

            # ---- attention: pair-major; exp on ScalarE paces the loop ----
            # QK proj of pair 1 is folded into pair 0's qc boundaries (the
            # PSUM "av" slot is free there and the PE has slack under exp).
            Exp = mybir.ActivationFunctionType.Exp

            for pair in range(2):
                for qc in range(NQC):
                    q_sl = slice(qc * SC, (qc + 1) * SC)

                    heads = []
                    for hh in range(2):
                        h = 2 * pair + hh
                        if h == 0:
                            heads.append((q0r[:, :], k0r[:, :]))
                        else:
                            po = hh * D
                            heads.append((QT_sb[po:po + D, pair, :],
                                          KT_sb[po:po + D, pair, :]))

                    # av is allocated lazily (first av_mm) so the qc-boundary
                    # proj chunk can use the shared "av" PSUM slot first
                    avh = []
                    pts = {}

                    def scores(j):
                        if j % 4 == 0:
                            pts[j // 4] = ptpool.tile(
                                [P, 4, 2, SC], BF, name="PT", tag="PT")
                        s_ps = psum.tile([P, 2, SC], F32, name="s_ps",
                                         tag="s_ps", bufs=2)
                        for par in range(2):
                            for hh in range(2):
                                qh, kh = heads[hh]
                                kt64 = 2 * j + par
                                nc.tensor.matmul(
                                    out=s_ps[par * D:(par + 1) * D, hh, :],
                                    lhsT=kh[:, kt64 * D:(kt64 + 1) * D],
                                    rhs=qh[:, q_sl],
                                    start=True, stop=True)
                        nc.scalar.activation(
                            out=pts[j // 4][:, j % 4, :, :], in_=s_ps[:, :, :],
                            func=Exp, scale=0.125)

                    def av_mm(j):
                        if not avh:
                            avh.append(psum.tile([P, 4, SC], F32, name="av",
                                                 tag="av", bufs=1))
                        av = avh[0]
                        # (hh, par) -> bank 2*hh+par; concurrent pairs are
                        # always disjoint in row group AND psum bank
                        for hh, par in ((0, 0), (1, 1), (0, 1), (1, 0)):
                            h = 2 * pair + hh
                            pp = slice(par * D, (par + 1) * D)
                            nc.tensor.matmul(
                                out=av[0:D + 1, 2 * hh + par, :],
                                lhsT=V_sb[pp, j, h, :],
                                rhs=pts[j // 4][pp, j % 4, hh, :],
                                start=(j == 0), stop=(j == NKTP - 1),
                                skip_group_check=True)

                    # prologue: 4 scores slabs; proj chunk hides under exp
                    for j in range(4):
                        scores(j)
                    if pair == 0:
                        qk_proj_chunk(nc, psum, wqkvT_sb, hT_sb, QT_sb, KT_sb,
                                      qkvbc_sb, qc)
                    for j in range(4):
                        av_mm(j)
                        if j >= 2:
                            scores(j + 2)
                    for j in range(4, NKTP):
                        if j + 2 < NKTP:
                            scores(j + 2)
                        av_mm(j)

                    # ---- drain: O^T (unnormalized) + 1/denominator ----
                    av = avh[0]
                    den2 = work.tile([1, 2, SC], F32, name="den2", tag="den2")
                    rcpf = work.tile([1, 2, SC], F32, name="rcpf", tag="rcpf")
                    rcpb = work.tile([1, 2, SC], BF, name="rcpb", tag="rcpb")
                    # engines may read only ONE psum input per op: copy the
                    # even-parity bank out first, then accumulate the odd one
                    for hh in range(2):
                        osum = work.tile([D, SC], F32, name="osum", tag="osum")
                        nc.vector.tensor_copy(out=osum[:, :],
                                              in_=av[0:D, 2 * hh, :])
                        nc.vector.tensor_add(
                            out=OT_sb[hh * D:(hh + 1) * D, pair, q_sl],
                            in0=osum[:, :], in1=av[0:D, 2 * hh + 1, :])
                        nc.vector.tensor_copy(
                            out=den2[0:1, hh, :], in_=av[D:D + 1, 2 * hh, :])
                        nc.vector.tensor_add(
                            out=den2[0:1, hh, :], in0=den2[0:1, hh, :],
                            in1=av[D:D + 1, 2 * hh + 1, :])
                    nc.vector.reciprocal_approx_fast(
                        out=rcpf[0:1, :, :], in_=den2[0:1, :, :])
                    nc.vector.tensor_copy(out=rcpb[0:1, :, :], in_=rcpf[0:1, :, :])
                    # partition-broadcast via DRAM bounce (SBUF APs cannot
                    # step-0 over partitions)
                    rcp_dr = dram.tile([1, 2, SC], BF, name="rcp_dr",
                                       tag="rcp_dr", bufs=4)
                    nc.sync.dma_start(out=rcp_dr[:, :, :], in_=rcpb[0:1, :, :])
                    rbc = work.tile([P, SC], BF, name="rbc", tag="rbc", bufs=2)
                    for hh in range(2):
                        nc.gpsimd.dma_start(
                            out=rbc[hh * D:(hh + 1) * D, :],
                            in_=rcp_dr[0:1, hh, :].to_broadcast([D, SC]))
                    nc.vector.tensor_mul(
                        out=OT_sb[:, pair, q_sl], in0=OT_sb[:, pair, q_sl],
                        in1=rbc[:, :])

            # ---- out projection tail: one 4-bank PSUM tile, halves
            # ping-pong by st parity; drains split across ACT (idle) and DVE
            yt = psum.tile([P, 4, SC], F32, name="yt", tag="av", bufs=1)
            for st in range(NST):
                half = st % 2
                for oc in range(DIM // SC):
                    for et in range(E // P):
                        nc.tensor.matmul(
                            out=yt[:, 2 * half + oc, :],
                            lhsT=OT_sb[:, et, st * P:(st + 1) * P],
                            rhs=woutT_sb[:, et, oc * SC:(oc + 1) * SC],
                            start=(et == 0), stop=(et == E // P - 1))
                y_sb = work.tile([P, DIM], F32, name="y_sb", tag="y_sb")
                nc.scalar.copy(out=y_sb[:, 0:SC], in_=yt[:, 2 * half, :])
                nc.vector.tensor_copy(out=y_sb[:, SC:DIM],
                                      in_=yt[:, 2 * half + 1, :])
                nc.sync.dma_start(
                    out=y_d.ap()[st * P:(st + 1) * P, :], in_=y_sb[:, :])

    return nc


def qk_proj_chunk(nc, psum, wqkvT_sb, hT_sb, QT_sb, KT_sb, qkvbc_sb, qc):
    """One quarter of pair 1's Q^T/K^T projection, run in the freed "av"
    PSUM slot at a qc boundary of pair 0's attention."""
    F32_ = mybir.dt.float32
    which = qc // 2
    dst = QT_sb if which == 0 else KT_sb
    avp = psum.tile([P, 4, SC], F32_, name="avp", tag="av", bufs=1)
    for k2 in range(2):
        scn = (qc % 2) * 2 + k2
        s_sl = slice(scn * SC, (scn + 1) * SC)
        ecols = slice(which * E + P, which * E + 2 * P)
        for kt in range(DIM // P):
            nc.tensor.matmul(
                out=avp[:, k2, :],
                lhsT=wqkvT_sb[:, kt, ecols],
                rhs=hT_sb[:, kt, s_sl],
                start=(kt == 0), stop=(kt == DIM // P - 1))
        nc.vector.tensor_scalar_add(
            out=dst[:, 1, s_sl], in0=avp[:, k2, :],
            scalar1=qkvbc_sb[:, which * 2 + 1, :])


def _shard_inputs(hidden_states, cos, sin, qkv_w, qkv_b, out_w):
    """Host-side prep: per-core transposed bf16 shards."""
    hs = np.asarray(hidden_states, dtype=np.float32)
    cos = np.asarray(cos, dtype=np.float32)
    sin = np.asarray(sin, dtype=np.float32)
    qkv_w = np.asarray(qkv_w, dtype=np.float32)
    qkv_b = np.asarray(qkv_b, dtype=np.float32)
    out_w = np.asarray(out_w, dtype=np.float32)

    def bf(x):
        return np.ascontiguousarray(x).astype(_BF_NP)

    hT_b = [bf(hs[b].T) for b in range(B)]
    in_maps = []
    for core in range(NCORES):
        b, g = divmod(core, GROUPS)
        e0 = E * g
        wq = qkv_w[e0:e0 + E]
        wk = qkv_w[H * D + e0:H * D + e0 + E]
        wv = qkv_w[2 * H * D + e0:2 * H * D + e0 + E]
        wqkvT = bf(np.concatenate([wq, wk, wv], axis=0).T)      # [DIM, 768]
        qkvb = bf(np.concatenate([
            qkv_b[e0:e0 + E], qkv_b[H * D + e0:H * D + e0 + E],
            qkv_b[2 * H * D + e0:2 * H * D + e0 + E]])[None, :])  # [1, 768]
        qkvb_col = np.ascontiguousarray(np.concatenate([
            qkv_b[e0:e0 + E], qkv_b[H * D + e0:H * D + e0 + E]]
        )[:, None].astype(np.float32))  # [512, 1] q|k bias as column
        woutT = bf(out_w[:, e0:e0 + E].T)                        # [256, DIM]
        if g == 0:
            c = cos[b].T
            sgn = np.where(np.arange(D) % 2 == 0, -1.0, 1.0)[:, None].astype(np.float32)
            s_ = sin[b].T * sgn
        else:
            c = np.ones((D, S), np.float32)
            s_ = np.zeros((D, S), np.float32)
        perm = np.zeros((D, D), np.float32)
        perm[np.arange(D), np.arange(D) ^ 1] = 1.0
        in_maps.append({
            "hT": hT_b[b],
            "wqkvT": wqkvT,
            "qkvb": qkvb,
            "qkvb_col": qkvb_col,
            "woutT": woutT,
            "cos_t": bf(c),
            "sin_t": bf(s_),
            "perm": bf(perm),
        })
    return in_maps


_last_results = None


def _ensure_axon_hooks():
    """run_bass_kernel_spmd imports antenv.axon_hooks when BASS_TRACE is set;
    this image's antenv lacks that module. Provide a no-op stand-in (hook=None
    -> tracing is skipped, run proceeds) so a stray BASS_TRACE can't crash."""
    try:
        import antenv.axon_hooks  # noqa: F401
    except ImportError:
        import sys as _sys
        import types as _types
        try:
            import antenv
        except ImportError:
            return
        mod = _types.ModuleType("antenv.axon_hooks")
        _state = {"hook": None}
        mod.set_axon_ntff_profile_hook = lambda h: _state.__setitem__("hook", h)
        mod.get_axon_ntff_profile_hook = lambda: _state["hook"]
        _sys.modules["antenv.axon_hooks"] = mod
        antenv.axon_hooks = mod


def kernel(hidden_states, cos, sin, qkv_w, qkv_b, out_w, out_b):
    global _last_results
    _ensure_axon_hooks()
    in_maps = _shard_inputs(hidden_states, cos, sin, qkv_w, qkv_b, out_w)
    nc = _build_nc()
    nc.compile()  # Bacc defers register allocation to compile()
    res = run_bass_kernel_spmd(nc, in_maps, core_ids=list(range(NCORES)))
    _last_results = res
    ys = [np.asarray(res.results[c]["y"], dtype=np.float32) for c in range(NCORES)]
    out_b = np.asarray(out_b, dtype=np.float32)
    out = np.stack([
        ys[0] + ys[1] + ys[2] + ys[3] + out_b[None, :],
        ys[4] + ys[5] + ys[6] + ys[7] + out_b[None, :],
    ])
    return out.astype(np.float32)


if __name__ == "__main__":
    nc = _build_nc()
    n_inst = sum(len(bb.instructions) for f in nc.m.functions for bb in f.blocks)
    print(f"built nc with {n_inst} instructions")
